# revision 12
# baseline (speedup 1.0000x reference)
"""Trainium2 Bass kernel for nn_MeanField (CRF mean-field mask refinement).

Contract: kernel(**inputs) takes the FULL inputs (B=16 images) as numpy
arrays, shards batch across 8 NeuronCores (2 images/core, pure data
parallel), runs a Bass/Tile kernel per core, and returns
(rgb_mask, batch_ious) matching reference().

Device algorithm (per core, 2 images):
  - Mean field runs in logit space: with L = log(x/(1-x)),
    one iteration is  L <- clip( sum_j k_j * L[.+o_j], -ln9, +ln9 ),
    algebraically identical to the reference's exp/log formulation
    (up to the negligible 1e-6 term in the normalizer).
  - Image layout: row r of image m lives at SBUF partition r>>1, block
    b = 2*m + (r&1), so a 256-row image fits the 128 partitions. Row
    shifts become partition shifts, done on the TensorEngine with 0/1
    shift matrices accumulating into PSUM; column shifts are free-axis
    AP offsets (2-col zero pads between blocks).
  - Affinity maps k_o = exp(-50*ssd_o) are built once on device from
    fm = feature+10 (host adds the +10); the depth-weighted variant
    multiplies the maps by exp(-50*dd^2) in place between the two runs.
  - Symmetry k_{-o}[i] = k_o[i-o] lets every mean-field term be an
    UNSHIFTED elementwise product G_o = k_{-o} * L (VectorE) followed by
    a shifted PSUM accumulation s += shift_{+o}(G_o) (TensorE).
"""

import numpy as np

# ---------------- problem constants (hardcoded; spec is fixed) -------------
B, C, H, W = 16, 3, 256, 256
NCORES = 8
M = B // NCORES            # images per core
NUM_ITER = 10
LO, HI = 0.1, 0.9
NEGINV2Z2 = -50.0          # -1/(2*zeta^2), zeta=0.1
LMAX = float(np.float32(np.log(np.float32(0.9)) - np.log(np.float32(0.1))))
LMIN = -LMAX

# ---------------- on-chip layout ------------------------------------------
P = 128
NB = 4            # blocks per SBUF tile: b = 2*m + (r&1)
BST = 258         # block stride: 2 shared pad cols + 256 body
SW = 2 + NB * BST + 2      # 1036: leading 2-pad, 4 blocks, trailing 2-pad
SBS = [2 + BST * b for b in range(NB)]    # body start col of block b
PST = 512         # PSUM block stride (one 2KB bank per block)
PSW = NB * PST    # 2048

OFFS = [(dy, dx) for dy in range(-2, 3) for dx in range(-2, 3)
        if not (dy == 0 and dx == 0)]
POS = [o for o in OFFS if (o[0] > 0) or (o[0] == 0 and o[1] > 0)]
SLOT = {}
for _i, _o in enumerate(POS):
    SLOT[_o] = _i
    SLOT[(-_o[0], -_o[1])] = 12 + _i

USE_F32R = True   # fp32r matmuls for the mean-field accumulation (fast path)


def _pdelta(dy, par):
    """partition shift delta and source parity for a row shift of dy."""
    parp = (par + dy) & 1
    return (par + dy - parp) >> 1, parp


# ---------------- device program ------------------------------------------

def build_nc():
    import concourse.bacc as bacc
    import concourse.mybir as mybir
    from concourse.tile import TileContext

    f32 = mybir.dt.float32
    AL = mybir.AluOpType
    AF = mybir.ActivationFunctionType

    f32r = mybir.dt.float32r

    nc = bacc.Bacc("TRN2")
    fm_d = nc.dram_tensor("fm", (M, C, H, W), f32, kind="ExternalInput")
    dep_d = nc.dram_tensor("dep", (M, H, W), f32, kind="ExternalInput")
    lgt_d = nc.dram_tensor("lgt", (M, H, W), f32, kind="ExternalInput")
    wsh_d = nc.dram_tensor("wsh", (3, P, P), f32, kind="ExternalInput")
    wshr_d = nc.dram_tensor("wshr", (3, P, P), f32r, kind="ExternalInput")
    lr_d = nc.dram_tensor("lr", (M, H, W), f32, kind="ExternalOutput")
    ld_d = nc.dram_tensor("ld", (M, H, W), f32, kind="ExternalOutput")

    def body3(t, off=0):
        """(128, NB, 256) body view of a (128, SW) tile, col-shifted by off."""
        lo = 2 + off
        return t[:, lo:lo + NB * BST].rearrange("p (b c) -> p b c", c=BST)[:, :, :256]

    def mslot3(maps, j, off=0):
        base = j * SW + 2 + off
        return maps[:, base:base + NB * BST].rearrange(
            "p (b c) -> p b c", c=BST)[:, :, :256]

    def psum3(ps, off=0):
        r = ps.rearrange("p (b c) -> p b c", c=PST)
        return r[:, :, off:off + 256]

    def img3(t, mi):
        """(128, 2, 256) one-image view for DMA to/from (H, W) DRAM."""
        lo = 2 + 2 * BST * mi
        return t[:, lo:lo + 2 * BST].rearrange(
            "p (q c) -> p q c", c=BST)[:, :, :256]

    with TileContext(nc) as tc:
        frees = []

        def T(name, width=SW, space="SBUF", dtype=f32):
            t, fr = tc.tile([P, width], dtype, space=space, name=name)
            frees.append(fr)
            return t

        fmt = [T(f"fmt{c}") for c in range(C)]
        dep = T("dep")
        L0 = T("L0")
        La = T("La")
        Lb = T("Lb")
        G = [T(f"G{i}", dtype=f32r if USE_F32R else f32) for i in range(4)]
        scr = [T(f"scr{i}") for i in range(4)]   # d/sq/ds scratch
        xtr = [T(f"xtr{i}") for i in range(3)]   # extra ssd scratch
        wt = T("wt", width=3 * P)
        wtr = T("wtr", width=3 * P, dtype=f32r)
        maps = T("maps", width=24 * SW)
        ps = [T(f"ps{i}", width=PSW, space="PSUM") for i in range(2)]

        # ---- zero-init tiles whose pads are read via shifted APs ----
        for t in fmt + [dep]:
            nc.vector.memset(t[:, :], 0.0)
        for t in G:
            nc.vector.memset(t[:, :].bitcast(mybir.dt.uint32), 0)
        mp = maps.rearrange("p (j r) -> p j r", r=SW)
        for b in range(5):
            nc.vector.memset(mp[:, :, BST * b:BST * b + 2], 0.0)
        nc.vector.memset(mp[:, :, 2 + NB * BST:SW], 0.0)

        # ---- input DMAs ----
        for mi in range(M):
            for c in range(C):
                nc.sync.dma_start(
                    out=img3(fmt[c], mi),
                    in_=fm_d[mi, c].rearrange("(p q) c -> p q c", q=2))
            nc.sync.dma_start(out=img3(dep, mi),
                              in_=dep_d[mi].rearrange("(p q) c -> p q c", q=2))
            nc.sync.dma_start(out=img3(L0, mi),
                              in_=lgt_d[mi].rearrange("(p q) c -> p q c", q=2))
        for d in range(3):
            nc.sync.dma_start(out=wt[:, P * d:P * (d + 1)], in_=wsh_d[d])
            nc.sync.dma_start(out=wtr[:, P * d:P * (d + 1)], in_=wshr_d[d])

        def Wd(delta):
            return wt[:, P * (delta + 1):P * (delta + 2)]

        def Wdr(delta):
            return wtr[:, P * (delta + 1):P * (delta + 2)]

        ps_ctr = [0]

        def next_ps():
            ps_ctr[0] += 1
            return ps[ps_ctr[0] % 2]

        def pe_shift(src, dy, dx, width=256):
            """TensorE: shifted copy of src tile into a fresh PSUM tile.

            psum block b col j  <-  src block b' col (base + j), where for
            width=260 base = SBS[b']-2+dx (covers image cols -2..258), and
            for width=256 base = SBS[b']+dx (body cols only).
            """
            p = next_ps()
            for b in range(NB):
                par, m = b & 1, b >> 1
                d, parp = _pdelta(dy, par)
                bp = 2 * m + parp
                lo = SBS[bp] + dx - (2 if width == 260 else 0)
                nc.tensor.matmul(p[:, PST * b:PST * b + width], Wd(d),
                                 src[:, lo:lo + width], start=True, stop=True)
            return p

        # ---- build affinity maps: k_o = exp(-50 * ssd_o), o in POS ----
        dxs_for = {0: [1, 2], 1: [-2, -1, 0, 1, 2], 2: [-2, -1, 0, 1, 2]}
        ssd = [La, Lb, xtr[0], xtr[1], xtr[2]]   # scratch before MF runs
        d_t = [scr[0], scr[1]]
        sq_t = [scr[2], scr[3]]

        for dy in (0, 1, 2):
            dxs = dxs_for[dy]
            for c in range(C):
                psh = pe_shift(fmt[c], dy, 0, width=260) if dy > 0 else None
                for k, dx in enumerate(dxs):
                    src = psum3(psh, 2 + dx) if dy > 0 else body3(fmt[c], dx)
                    dt = d_t[k % 2]
                    nc.vector.tensor_tensor(body3(dt), src, body3(fmt[c]),
                                            op=AL.subtract)
                    if c == 0:
                        nc.scalar.square(body3(ssd[k]), body3(dt))
                    else:
                        st = sq_t[k % 2]
                        nc.scalar.square(body3(st), body3(dt))
                        nc.vector.tensor_tensor(body3(ssd[k]), body3(ssd[k]),
                                                body3(st), op=AL.add)
            for k, dx in enumerate(dxs):
                nc.scalar.activation(mslot3(maps, SLOT[(dy, dx)]),
                                     body3(ssd[k]), AF.Exp, scale=NEGINV2Z2)

        def shift_copy_neg(jsrc, jdst, dy, dx):
            """maps[jdst] = shift_{(-dy,-dx)}(maps[jsrc]) (k_{-o} from k_o)."""
            p = next_ps()
            for b in range(NB):
                par, m = b & 1, b >> 1
                d, parp = _pdelta(-dy, par)
                bp = 2 * m + parp
                lo = jsrc * SW + SBS[bp] - dx
                nc.tensor.matmul(p[:, PST * b:PST * b + 256], Wd(d),
                                 maps[:, lo:lo + 256], start=True, stop=True)
            nc.vector.tensor_copy(mslot3(maps, jdst), psum3(p))

        for o in POS:
            shift_copy_neg(SLOT[o], SLOT[(-o[0], -o[1])], o[0], o[1])

        # ---- mean-field runs ----

        def mf_run(out_dram):
            Lcur = L0
            for it in range(NUM_ITER):
                Lnext = La if it % 2 == 0 else Lb
                s = next_ps()
                for j, (dy, dx) in enumerate(OFFS):
                    g = G[j % 4]
                    nc.vector.tensor_tensor(
                        body3(g), mslot3(maps, SLOT[(-dy, -dx)]), body3(Lcur),
                        op=AL.mult)
                    first, last = (j == 0), (j == len(OFFS) - 1)
                    for b in range(NB):
                        par, m = b & 1, b >> 1
                        d, parp = _pdelta(dy, par)
                        bp = 2 * m + parp
                        lhsT = Wdr(d) if USE_F32R else Wd(d)
                        rhs = g[:, SBS[bp] + dx:SBS[bp] + dx + 256]
                        nc.tensor.matmul(s[:, PST * b:PST * b + 256], lhsT, rhs,
                                         start=first, stop=last)
                # center term (weight exactly 1) in exact fp32, then clip
                nc.vector.tensor_tensor(body3(Lnext), psum3(s), body3(Lcur),
                                        op=AL.add)
                nc.vector.tensor_scalar(body3(Lnext), body3(Lnext),
                                        LMIN, LMAX, op0=AL.max, op1=AL.min)
                Lcur = Lnext
            for mi in range(M):
                nc.sync.dma_start(
                    out=out_dram[mi].rearrange("(p q) c -> p q c", q=2),
                    in_=img3(Lcur, mi))

        mf_run(lr_d)

        # ---- depth weighting: maps *= exp(-50*dd^2), then refresh k_{-o} ----
        ds = scr[3]
        for dy in (0, 1, 2):
            dxs = dxs_for[dy]
            psh = pe_shift(dep, dy, 0, width=260) if dy > 0 else None
            for k, dx in enumerate(dxs):
                src = psum3(psh, 2 + dx) if dy > 0 else body3(dep, dx)
                dt = scr[k % 2]
                nc.vector.tensor_tensor(body3(dt), src, body3(dep),
                                        op=AL.subtract)
                nc.scalar.square(body3(scr[2]), body3(dt))
                nc.scalar.activation(body3(ds), body3(scr[2]), AF.Exp,
                                     scale=NEGINV2Z2)
                j = SLOT[(dy, dx)]
                nc.vector.tensor_tensor(mslot3(maps, j), mslot3(maps, j),
                                        body3(ds), op=AL.mult)
        for o in POS:
            shift_copy_neg(SLOT[o], SLOT[(-o[0], -o[1])], o[0], o[1])

        mf_run(ld_d)

        for fr in reversed(frees):
            fr()
    if not nc.is_finalized():
        nc.finalize()
    return nc


# ---------------- host wrapper --------------------------------------------

_NC_CACHE = {}


def _get_nc():
    if "nc" not in _NC_CACHE:
        _NC_CACHE["nc"] = build_nc()
    return _NC_CACHE["nc"]


def _shift_weights():
    w = np.zeros((3, P, P), dtype=np.float32)
    for d, delta in enumerate((-1, 0, 1)):
        for po in range(P):
            pi = po + delta
            if 0 <= pi < P:
                w[d, pi, po] = 1.0
    return w


def _in_maps(feature_map, seg, depth_map):
    fm = feature_map.astype(np.float32) + np.float32(10.0)
    x0 = np.clip(seg.astype(np.float64), LO, HI)
    lgt = (np.log(x0) - np.log1p(-x0)).astype(np.float32)
    dep = depth_map.astype(np.float32)
    wsh = _shift_weights()
    return [{"fm": np.ascontiguousarray(fm[M * i:M * (i + 1)]),
             "dep": np.ascontiguousarray(dep[M * i:M * (i + 1)]),
             "lgt": np.ascontiguousarray(lgt[M * i:M * (i + 1)]),
             "wsh": wsh, "wshr": wsh} for i in range(NCORES)]


def run_device(feature_map, seg, depth_map, **spmd_kwargs):
    """Run the Bass kernel on 8 cores; returns (Lrgb, Lrgbd) full batch."""
    from concourse.bass_utils import run_bass_kernel_spmd
    res = run_bass_kernel_spmd(_get_nc(), _in_maps(feature_map, seg, depth_map),
                               core_ids=list(range(NCORES)), **spmd_kwargs)
    Lr = np.concatenate([r["lr"] for r in res.results], axis=0)
    Ld = np.concatenate([r["ld"] for r in res.results], axis=0)
    return Lr, Ld, res


def kernel(feature_map, seg, depth_map, targets, sam_mask):
    feature_map = np.asarray(feature_map)
    seg = np.asarray(seg)
    depth_map = np.asarray(depth_map)
    targets = np.asarray(targets)
    sam_mask = np.asarray(sam_mask)

    Lr, Ld, _ = run_device(feature_map, seg, depth_map)

    rgb_mask = (Lr > 0).astype(np.float32)
    mask_rgb_depth = (Ld > 0).astype(np.float32)
    orig_mask = (seg > 0.5).astype(np.float32)

    t = targets != 0

    def iou_mean(pm):
        pi = pm != 0
        inter = np.sum(t & pi, axis=(1, 2)).astype(np.float32)
        union = np.sum(t | pi, axis=(1, 2)).astype(np.float32)
        return np.float32(np.mean(inter / (union + np.float32(1e-6))))

    batch_ious = np.stack([iou_mean(orig_mask), iou_mean(rgb_mask),
                           iou_mean(mask_rgb_depth), iou_mean(sam_mask)])
    return rgb_mask, batch_ious


# revision 14
# speedup vs baseline: 1.0158x; 1.0158x over previous
"""Trainium2 Bass kernel for nn_MeanField (CRF mean-field mask refinement).

Contract: kernel(**inputs) takes the FULL inputs (B=16 images) as numpy
arrays, shards batch across 8 NeuronCores (2 images/core, pure data
parallel), runs a Bass/Tile kernel per core, and returns
(rgb_mask, batch_ious) matching reference().

Device algorithm (per core, 2 images):
  - Mean field runs in logit space: with L = log(x/(1-x)),
    one iteration is  L <- clip( sum_j k_j * L[.+o_j], -ln9, +ln9 ),
    algebraically identical to the reference's exp/log formulation
    (up to the negligible 1e-6 term in the normalizer).
  - Image layout: row r of image m lives at SBUF partition r>>1, block
    b = 2*m + (r&1), so a 256-row image fits the 128 partitions. Row
    shifts become partition shifts, done on the TensorEngine with 0/1
    shift matrices accumulating into PSUM; column shifts are free-axis
    AP offsets (2-col zero pads between blocks).
  - Affinity maps k_o = exp(-50*ssd_o) are built once on device from
    fm = feature+10 (host adds the +10); the depth-weighted variant
    multiplies the maps by exp(-50*dd^2) in place between the two runs.
  - Symmetry k_{-o}[i] = k_o[i-o] lets every mean-field term be an
    UNSHIFTED elementwise product G_o = k_{-o} * L (VectorE) followed by
    a shifted PSUM accumulation s += shift_{+o}(G_o) (TensorE).
"""

import numpy as np

# ---------------- problem constants (hardcoded; spec is fixed) -------------
B, C, H, W = 16, 3, 256, 256
NCORES = 8
M = B // NCORES            # images per core
NUM_ITER = 10
LO, HI = 0.1, 0.9
NEGINV2Z2 = -50.0          # -1/(2*zeta^2), zeta=0.1
LMAX = float(np.float32(np.log(np.float32(0.9)) - np.log(np.float32(0.1))))
LMIN = -LMAX

# ---------------- on-chip layout ------------------------------------------
P = 128
NB = 4            # blocks per SBUF tile: b = 2*m + (r&1)
BST = 258         # block stride: 2 shared pad cols + 256 body
SW = 2 + NB * BST + 2      # 1036: leading 2-pad, 4 blocks, trailing 2-pad
SBS = [2 + BST * b for b in range(NB)]    # body start col of block b
PST = 512         # PSUM block stride (one 2KB bank per block)
PSW = NB * PST    # 2048

OFFS = [(dy, dx) for dy in range(-2, 3) for dx in range(-2, 3)
        if not (dy == 0 and dx == 0)]
POS = [o for o in OFFS if (o[0] > 0) or (o[0] == 0 and o[1] > 0)]
SLOT = {}
for _i, _o in enumerate(POS):
    SLOT[_o] = _i
    SLOT[(-_o[0], -_o[1])] = 12 + _i

USE_F32R = True   # fp32r matmuls for the mean-field accumulation (fast path)


def _pdelta(dy, par):
    """partition shift delta and source parity for a row shift of dy."""
    parp = (par + dy) & 1
    return (par + dy - parp) >> 1, parp


# ---------------- device program ------------------------------------------

def build_nc():
    import concourse.bacc as bacc
    import concourse.mybir as mybir
    from concourse.tile import TileContext

    f32 = mybir.dt.float32
    AL = mybir.AluOpType
    AF = mybir.ActivationFunctionType

    f32r = mybir.dt.float32r

    nc = bacc.Bacc("TRN2")
    fm_d = nc.dram_tensor("fm", (M, C, H, W), f32, kind="ExternalInput")
    dep_d = nc.dram_tensor("dep", (M, H, W), f32, kind="ExternalInput")
    lgt_d = nc.dram_tensor("lgt", (M, H, W), f32, kind="ExternalInput")
    wsh_d = nc.dram_tensor("wsh", (3, P, P), f32, kind="ExternalInput")
    wshr_d = nc.dram_tensor("wshr", (3, P, P), f32r, kind="ExternalInput")
    lr_d = nc.dram_tensor("lr", (M, H, W), f32, kind="ExternalOutput")
    ld_d = nc.dram_tensor("ld", (M, H, W), f32, kind="ExternalOutput")

    def body3(t, off=0):
        """(128, NB, 256) body view of a (128, SW) tile, col-shifted by off."""
        lo = 2 + off
        return t[:, lo:lo + NB * BST].rearrange("p (b c) -> p b c", c=BST)[:, :, :256]

    def mslot3(maps, j, off=0):
        base = j * SW + 2 + off
        return maps[:, base:base + NB * BST].rearrange(
            "p (b c) -> p b c", c=BST)[:, :, :256]

    def psum3(ps, off=0):
        r = ps.rearrange("p (b c) -> p b c", c=PST)
        return r[:, :, off:off + 256]

    def img3(t, mi):
        """(128, 2, 256) one-image view for DMA to/from (H, W) DRAM."""
        lo = 2 + 2 * BST * mi
        return t[:, lo:lo + 2 * BST].rearrange(
            "p (q c) -> p q c", c=BST)[:, :, :256]

    with TileContext(nc) as tc:
        frees = []

        def T(name, width=SW, space="SBUF", dtype=f32):
            t, fr = tc.tile([P, width], dtype, space=space, name=name)
            frees.append(fr)
            return t

        fmt = [T(f"fmt{c}") for c in range(C)]
        dep = T("dep")
        L0 = T("L0")
        La = T("La")
        Lb = T("Lb")
        G = [T(f"G{i}", dtype=f32r if USE_F32R else f32) for i in range(4)]
        scr = [T(f"scr{i}") for i in range(4)]   # d/sq/ds scratch
        xtr = [T(f"xtr{i}") for i in range(3)]   # extra ssd scratch
        wt = T("wt", width=3 * P)
        wtr = T("wtr", width=3 * P, dtype=f32r)
        maps = T("maps", width=24 * SW)
        ps = [T(f"ps{i}", width=PSW, space="PSUM") for i in range(2)]

        # ---- zero-init tiles whose pads are read via shifted APs ----
        for t in fmt + [dep]:
            nc.vector.memset(t[:, :], 0.0)
        for t in G:
            nc.vector.memset(t[:, :].bitcast(mybir.dt.uint32), 0)
        mp = maps.rearrange("p (j r) -> p j r", r=SW)
        for b in range(5):
            nc.vector.memset(mp[:, :, BST * b:BST * b + 2], 0.0)
        nc.vector.memset(mp[:, :, 2 + NB * BST:SW], 0.0)

        # ---- input DMAs ----
        for mi in range(M):
            for c in range(C):
                nc.sync.dma_start(
                    out=img3(fmt[c], mi),
                    in_=fm_d[mi, c].rearrange("(p q) c -> p q c", q=2))
            nc.sync.dma_start(out=img3(dep, mi),
                              in_=dep_d[mi].rearrange("(p q) c -> p q c", q=2))
            nc.sync.dma_start(out=img3(L0, mi),
                              in_=lgt_d[mi].rearrange("(p q) c -> p q c", q=2))
        for d in range(3):
            nc.sync.dma_start(out=wt[:, P * d:P * (d + 1)], in_=wsh_d[d])
            nc.sync.dma_start(out=wtr[:, P * d:P * (d + 1)], in_=wshr_d[d])

        def Wd(delta):
            return wt[:, P * (delta + 1):P * (delta + 2)]

        def Wdr(delta):
            return wtr[:, P * (delta + 1):P * (delta + 2)]

        ps_ctr = [0]

        def next_ps():
            ps_ctr[0] += 1
            return ps[ps_ctr[0] % 2]

        def pe_shift(src, dy, dx, width=256):
            """TensorE: shifted copy of src tile into a fresh PSUM tile.

            psum block b col j  <-  src block b' col (base + j), where for
            width=260 base = SBS[b']-2+dx (covers image cols -2..258), and
            for width=256 base = SBS[b']+dx (body cols only).
            """
            p = next_ps()
            for b in range(NB):
                par, m = b & 1, b >> 1
                d, parp = _pdelta(dy, par)
                bp = 2 * m + parp
                lo = SBS[bp] + dx - (2 if width == 260 else 0)
                nc.tensor.matmul(p[:, PST * b:PST * b + width], Wd(d),
                                 src[:, lo:lo + width], start=True, stop=True)
            return p

        # ---- build affinity maps: k_o = exp(-50 * ssd_o), o in POS ----
        dxs_for = {0: [1, 2], 1: [-2, -1, 0, 1, 2], 2: [-2, -1, 0, 1, 2]}
        ssd = [La, Lb, xtr[0], xtr[1], xtr[2]]   # scratch before MF runs
        d_t = [scr[0], scr[1]]
        sq_t = [scr[2], scr[3]]

        for dy in (0, 1, 2):
            dxs = dxs_for[dy]
            for c in range(C):
                psh = pe_shift(fmt[c], dy, 0, width=260) if dy > 0 else None
                for k, dx in enumerate(dxs):
                    src = psum3(psh, 2 + dx) if dy > 0 else body3(fmt[c], dx)
                    dt = d_t[k % 2]
                    nc.vector.tensor_tensor(body3(dt), src, body3(fmt[c]),
                                            op=AL.subtract)
                    if c == 0:
                        nc.scalar.square(body3(ssd[k]), body3(dt))
                    else:
                        st = sq_t[k % 2]
                        nc.scalar.square(body3(st), body3(dt))
                        nc.vector.tensor_tensor(body3(ssd[k]), body3(ssd[k]),
                                                body3(st), op=AL.add)
            for k, dx in enumerate(dxs):
                nc.scalar.activation(mslot3(maps, SLOT[(dy, dx)]),
                                     body3(ssd[k]), AF.Exp, scale=NEGINV2Z2)

        def shift_copy_neg(jsrc, jdst, dy, dx):
            """maps[jdst] = shift_{(-dy,-dx)}(maps[jsrc]) (k_{-o} from k_o)."""
            p = next_ps()
            for b in range(NB):
                par, m = b & 1, b >> 1
                d, parp = _pdelta(-dy, par)
                bp = 2 * m + parp
                lo = jsrc * SW + SBS[bp] - dx
                nc.tensor.matmul(p[:, PST * b:PST * b + 256], Wd(d),
                                 maps[:, lo:lo + 256], start=True, stop=True)
            nc.scalar.copy(mslot3(maps, jdst), psum3(p))

        for o in POS:
            shift_copy_neg(SLOT[o], SLOT[(-o[0], -o[1])], o[0], o[1])

        # ---- mean-field runs ----

        def mf_run(out_dram):
            Lcur = L0
            for it in range(NUM_ITER):
                Lnext = La if it % 2 == 0 else Lb
                s = next_ps()
                # center term (weight exactly 1) as an EXACT fp32 identity
                # matmul; it leads the accumulation group (start=True).
                for b in range(NB):
                    nc.tensor.matmul(s[:, PST * b:PST * b + 256], Wd(0),
                                     body3(Lcur)[:, b, :], start=True,
                                     stop=False)
                for j, (dy, dx) in enumerate(OFFS):
                    g = G[j % 4]
                    nc.vector.tensor_tensor(
                        body3(g), mslot3(maps, SLOT[(-dy, -dx)]), body3(Lcur),
                        op=AL.mult)
                    last = (j == len(OFFS) - 1)
                    for b in range(NB):
                        par, m = b & 1, b >> 1
                        d, parp = _pdelta(dy, par)
                        bp = 2 * m + parp
                        lhsT = Wdr(d) if USE_F32R else Wd(d)
                        rhs = g[:, SBS[bp] + dx:SBS[bp] + dx + 256]
                        nc.tensor.matmul(s[:, PST * b:PST * b + 256], lhsT, rhs,
                                         start=False, stop=last)
                nc.vector.tensor_scalar(body3(Lnext), psum3(s),
                                        LMIN, LMAX, op0=AL.max, op1=AL.min)
                Lcur = Lnext
            for mi in range(M):
                nc.sync.dma_start(
                    out=out_dram[mi].rearrange("(p q) c -> p q c", q=2),
                    in_=img3(Lcur, mi))

        mf_run(lr_d)

        # ---- depth weighting: maps *= exp(-50*dd^2), then refresh k_{-o} ----
        ds = scr[3]
        for dy in (0, 1, 2):
            dxs = dxs_for[dy]
            psh = pe_shift(dep, dy, 0, width=260) if dy > 0 else None
            for k, dx in enumerate(dxs):
                src = psum3(psh, 2 + dx) if dy > 0 else body3(dep, dx)
                dt = scr[k % 2]
                nc.vector.tensor_tensor(body3(dt), src, body3(dep),
                                        op=AL.subtract)
                nc.scalar.square(body3(scr[2]), body3(dt))
                nc.scalar.activation(body3(ds), body3(scr[2]), AF.Exp,
                                     scale=NEGINV2Z2)
                j = SLOT[(dy, dx)]
                nc.vector.tensor_tensor(mslot3(maps, j), mslot3(maps, j),
                                        body3(ds), op=AL.mult)
        for o in POS:
            shift_copy_neg(SLOT[o], SLOT[(-o[0], -o[1])], o[0], o[1])

        mf_run(ld_d)

        for fr in reversed(frees):
            fr()
    if not nc.is_finalized():
        nc.finalize()
    return nc


# ---------------- host wrapper --------------------------------------------

_NC_CACHE = {}


def _get_nc():
    if "nc" not in _NC_CACHE:
        _NC_CACHE["nc"] = build_nc()
    return _NC_CACHE["nc"]


def _shift_weights():
    w = np.zeros((3, P, P), dtype=np.float32)
    for d, delta in enumerate((-1, 0, 1)):
        for po in range(P):
            pi = po + delta
            if 0 <= pi < P:
                w[d, pi, po] = 1.0
    return w


def _in_maps(feature_map, seg, depth_map):
    fm = feature_map.astype(np.float32) + np.float32(10.0)
    x0 = np.clip(seg.astype(np.float64), LO, HI)
    lgt = (np.log(x0) - np.log1p(-x0)).astype(np.float32)
    dep = depth_map.astype(np.float32)
    wsh = _shift_weights()
    return [{"fm": np.ascontiguousarray(fm[M * i:M * (i + 1)]),
             "dep": np.ascontiguousarray(dep[M * i:M * (i + 1)]),
             "lgt": np.ascontiguousarray(lgt[M * i:M * (i + 1)]),
             "wsh": wsh, "wshr": wsh} for i in range(NCORES)]


def run_device(feature_map, seg, depth_map, **spmd_kwargs):
    """Run the Bass kernel on 8 cores; returns (Lrgb, Lrgbd) full batch."""
    from concourse.bass_utils import run_bass_kernel_spmd
    res = run_bass_kernel_spmd(_get_nc(), _in_maps(feature_map, seg, depth_map),
                               core_ids=list(range(NCORES)), **spmd_kwargs)
    Lr = np.concatenate([r["lr"] for r in res.results], axis=0)
    Ld = np.concatenate([r["ld"] for r in res.results], axis=0)
    return Lr, Ld, res


def kernel(feature_map, seg, depth_map, targets, sam_mask):
    feature_map = np.asarray(feature_map)
    seg = np.asarray(seg)
    depth_map = np.asarray(depth_map)
    targets = np.asarray(targets)
    sam_mask = np.asarray(sam_mask)

    Lr, Ld, _ = run_device(feature_map, seg, depth_map)

    rgb_mask = (Lr > 0).astype(np.float32)
    mask_rgb_depth = (Ld > 0).astype(np.float32)
    orig_mask = (seg > 0.5).astype(np.float32)

    t = targets != 0

    def iou_mean(pm):
        pi = pm != 0
        inter = np.sum(t & pi, axis=(1, 2)).astype(np.float32)
        union = np.sum(t | pi, axis=(1, 2)).astype(np.float32)
        return np.float32(np.mean(inter / (union + np.float32(1e-6))))

    batch_ious = np.stack([iou_mean(orig_mask), iou_mean(rgb_mask),
                           iou_mean(mask_rgb_depth), iou_mean(sam_mask)])
    return rgb_mask, batch_ious
